# revision 55
# baseline (speedup 1.0000x reference)
"""Causal multi-head attention (B=2, T=2048, DIM=1024, H=16) on 8 TRN2 cores.

Sharding: core c handles batch b = c // 4 and head-group g = c % 4 (4 heads,
head-dim slice of 256).  Each core computes QKV projections for its heads,
causal attention, and a partial output projection y_partial = o_g @ wo[:, g].T
of shape (2048, 1024).  Host sums the 4 partials per batch (the tensor-parallel
all-reduce, done on host as the unshard step).

All matmuls run in bfloat16 (full PE rate at any tile size; fp32r pays a 4x
penalty below 256 moving columns, which hits the causal diagonal tiles).
Inputs are pre-rounded to bf16 on host; end-to-end error vs the fp32
reference is ~2e-3, well inside the 2e-2 gate.

The kernel is a software pipeline over the 4 token-quarters so the scalar
(ACT) engine's exp work -- the single largest non-PE cost, ~63us -- overlaps
the PE-only projection and output-projection phases instead of running after
them:

    [proj Q0 | attn G0 | proj Q1 | y G0 | attn G1 | proj Q2 | y G1 | ...]

Attention group G only needs k/v tiles 0..4G+3, i.e. quarters <= G, so each
group runs as soon as its quarter is projected.  Projections are emitted at
half-quarter (256-token) granularity so the first matmul starts ~3us after
launch instead of waiting for a full 2MB x-quarter DMA.

Device layout (T=2048 tokens of one batch, DH=256 head dims of one group):
  xsb  [128, 8, T]   x with the DIM contraction split into 8 chunks of 128
  qT/kT[128, 2, T]   per pair p of 2 heads; partitions = 2x64 head dims
  v    [128, 16, 4, 65]  [t-tile, k-in-tile, head, head-dim + ones column]
  scores sT[k, q] via matmul(lhsT=kT, rhs=qT); softmax without max-subtraction
  (scores ~N(0,1)); denominator accumulated by the ones column of v during
  attn@v; normalization applied via a DRAM-bounce broadcast of 1/denom
  (PE outer-product broadcast for the final, tail-critical pair).
"""

import sys

sys.path.insert(0, "/opt/trn_rl_repo")

import numpy as np

B, T, DIM, H = 2, 2048, 1024, 16
HD = DIM // H          # 64
NCORES = 8
GROUPS = 4             # head-groups (one per core pair-of-batches)
GH = H // GROUPS       # 4 heads per group
DH = GH * HD           # 256 head dims per group
NPAIR = 2              # pairs of heads per group (2 heads = 128 partitions)
TT = T // 128          # 16 t-tiles
TG = T // 512          # 4 q-groups of 512


def _build_program(loop=1):
    import concourse.bass as bass
    import concourse.tile as tile
    from concourse import bacc, mybir

    F32 = mybir.dt.float32
    BF = mybir.dt.bfloat16
    AF = mybir.ActivationFunctionType

    nc = bacc.Bacc("TRN2", target_bir_lowering=False, debug=False,
                   num_devices=NCORES)

    xt_d = nc.dram_tensor("xt", [DIM, T], BF, kind="ExternalInput")
    wqt_d = nc.dram_tensor("wqt", [DIM, DH], BF, kind="ExternalInput")
    wkt_d = nc.dram_tensor("wkt", [DIM, DH], BF, kind="ExternalInput")
    wvt_d = nc.dram_tensor("wvt", [DIM, DH], BF, kind="ExternalInput")
    wot_d = nc.dram_tensor("wot", [DH, DIM], BF, kind="ExternalInput")
    y_d = nc.dram_tensor("y", [T, DIM], BF, kind="ExternalOutput")

    KO = DIM // 128  # 8 contraction chunks

    with tile.TileContext(nc) as tc:
        with (
            tc.tile_pool(name="singles", bufs=1) as singles,
            tc.tile_pool(name="workp", bufs=6) as workp,
            tc.tile_pool(name="worky", bufs=6) as worky,
            tc.tile_pool(name="tiny", bufs=4) as tiny,
            tc.tile_pool(name="ps2", bufs=2, space="PSUM") as ps2,
            tc.tile_pool(name="psa", bufs=2, space="PSUM") as psa,
            tc.tile_pool(name="psb", bufs=2, space="PSUM") as psb,
            tc.tile_pool(name="dramp", bufs=2, space="DRAM") as dramp,
        ):
            # ---- persistent SBUF tensors ----
            qT = singles.tile([128, NPAIR, T], BF)
            kT = singles.tile([128, NPAIR, T], BF)
            v = singles.tile([128, TT, GH, HD + 1], BF)
            oT = singles.tile([128, NPAIR, T], BF)

            mask01 = singles.tile([128, 128], BF)
            nc.gpsimd.memset(mask01[:], 1.0)
            # keep 1 where q - k >= 0 (k on partitions, q on free), else 0
            nc.gpsimd.affine_select(
                out=mask01[:], in_=mask01[:],
                compare_op=mybir.AluOpType.is_ge, fill=0.0,
                base=0, pattern=[[1, 128]], channel_multiplier=-1,
            )
            ones1 = singles.tile([1, HD], BF)
            nc.vector.memset(ones1[:], 1.0)
            # ones column of v (denominator accumulator)
            for hh in range(GH):
                nc.vector.memset(v[:, :, hh, HD:HD + 1], 1.0)
            # warm the ACT exp table during the initial DMA
            dummyf = singles.tile([128, 1], F32)
            nc.vector.memset(dummyf[:], 1.0)
            dummy = singles.tile([128, 1], F32)
            nc.scalar.activation(dummy[:], dummyf[:], AF.Exp)

            # ---- device-side repetition for timing (loop > 1) ----
            for _it in range(loop):
              with (tc.tile_pool(name=f"wpool{_it}", bufs=1) as wpool,):
                xt_r = xt_d.rearrange("(ko p) t -> p ko t", p=128)
                wqt_sb = wpool.tile([128, KO, DH], BF)
                wkt_sb = wpool.tile([128, KO, DH], BF)
                wvt_sb = wpool.tile([128, KO, DH], BF)
                wot_sb = wpool.tile([128, DH // 128, DIM], BF)
                xsb = wpool.tile([128, KO, T], BF)
                wqt_r = wqt_d.rearrange("(ko p) d -> p ko d", p=128)

                # DMA order = need order: wqt + first x half gate the first
                # matmul; later halves stream in behind the compute.
                nc.sync.dma_start(wqt_sb[:, 0:4, :], wqt_r[:, 0:4, :])
                nc.sync.dma_start(xsb[:, 0:2, 0:256], xt_r[:, 0:2, 0:256])
                nc.sync.dma_start(wqt_sb[:, 4:8, :], wqt_r[:, 4:8, :])
                nc.sync.dma_start(xsb[:, 2:4, 0:256], xt_r[:, 2:4, 0:256])
                nc.sync.dma_start(xsb[:, 4:6, 0:256], xt_r[:, 4:6, 0:256])
                nc.sync.dma_start(xsb[:, 6:8, 0:256], xt_r[:, 6:8, 0:256])
                nc.sync.dma_start(wkt_sb,
                                  wkt_d.rearrange("(ko p) d -> p ko d", p=128))
                nc.sync.dma_start(wvt_sb,
                                  wvt_d.rearrange("(ko p) d -> p ko d", p=128))
                for h in range(1, 8):
                    hs = slice(256 * h, 256 * (h + 1))
                    nc.sync.dma_start(xsb[:, :, hs], xt_r[:, :, hs])
                nc.sync.dma_start(wot_sb,
                                  wot_d.rearrange("(ko p) j -> p ko j", p=128))

                _tailctx = {}

                # ---- filler work units (one unit ~ 850ns of PE time) ----
                # Emitted between attention tiles so the PE fills the per-tile
                # deficit vs the ACT engine's exp (which is ~20% slower per
                # attention tile than the PE's scores+AV matmuls).
                _qk_acc = {}

                def make_qk_unit(w_sb, dst, h, p, key):
                    hs = slice(256 * h, 256 * (h + 1))

                    def f():
                        if p == 0:
                            _qk_acc[key] = psb.tile(
                                [128, 512], F32, tag="pacc",
                                name=f"acc_{key}_{_it}")
                        acc = _qk_acc[key]
                        for ko in range(KO):
                            nc.tensor.matmul(
                                acc[:, 256 * p:256 * (p + 1)],
                                w_sb[:, ko, 128 * p:128 * (p + 1)],
                                xsb[:, ko, hs],
                                start=(ko == 0), stop=(ko == KO - 1),
                            )
                        if p == NPAIR - 1:
                            nc.vector.tensor_copy(
                                dst[:, :, hs],
                                acc[:].rearrange("par (p t) -> par p t",
                                                 p=NPAIR))
                    return f

                def make_v_unit(tt):
                    def f():
                        acc = psb.tile([128, DH], F32, tag="pacc",
                                       name=f"vacc_{tt}_{_it}")
                        for ko in range(KO):
                            nc.tensor.matmul(
                                acc[:],
                                xsb[:, ko, 128 * tt:128 * (tt + 1)],
                                wvt_sb[:, ko, :],
                                start=(ko == 0), stop=(ko == KO - 1),
                            )
                        nc.vector.tensor_copy(
                            v[:, tt, :, 0:HD],
                            acc[:].rearrange("p (h d) -> p h d", h=GH))
                    return f

                def proj_units(h):
                    """Six ~850ns units projecting tokens [256h, 256h+256)."""
                    return [
                        make_qk_unit(wqt_sb, qT, h, 0, f"q{h}"),
                        make_qk_unit(wqt_sb, qT, h, 1, f"q{h}"),
                        make_qk_unit(wkt_sb, kT, h, 0, f"k{h}"),
                        make_qk_unit(wkt_sb, kT, h, 1, f"k{h}"),
                        make_v_unit(2 * h),
                        make_v_unit(2 * h + 1),
                    ]

                def make_y_unit(tt, jh, drain_eng=None):
                    """Half an output-projection tile: 1-bank psum, drained
                    on DVE so the y work never queues behind pending exps on
                    ACT (which would stall the shared psum rotation)."""
                    def f():
                        acc = psb.tile([128, 512], F32, tag="pacc",
                                       name=f"yacc_{tt}_{jh}_{_it}")
                        for p in range(NPAIR):
                            nc.tensor.matmul(
                                acc[:],
                                oT[:, p, 128 * tt:128 * (tt + 1)],
                                wot_sb[:, p, 512 * jh:512 * (jh + 1)],
                                start=(p == 0), stop=(p == NPAIR - 1),
                            )
                        ysb = worky.tile([128, 512], BF, tag="ysb")
                        if drain_eng is None:
                            nc.vector.tensor_copy(ysb[:], acc[:])
                        else:
                            drain_eng.copy(ysb[:], acc[:])
                        nc.sync.dma_start(
                            y_d[128 * tt:128 * (tt + 1),
                                512 * jh:512 * (jh + 1)], ysb[:])
                    return f

                def attn_norm(p, G, oA, oB):
                    """Softmax denominator normalization for pair p of group
                    G: o /= denom with 1/denom broadcast across partitions."""
                    qsl = slice(512 * G, 512 * (G + 1))
                    last = (p == NPAIR - 1 and G == TG - 1)
                    if last:
                        # reciprocals first: they gate the PE broadcast.
                        # Everything is halved so the first tail mul starts
                        # one DVE-op-latency earlier.
                        recA = tiny.tile([1, 512], BF, tag="recA")
                        recB = tiny.tile([1, 512], BF, tag="recB")
                        rpAB = ps2.tile([128, 512], F32, tag="big",
                                        name=f"rpAB_{_it}")
                        rpSB = tiny.tile([128, 512], F32, tag="rpSB")
                        for hh in (slice(256, 512), slice(0, 256)):
                            with nc.allow_low_precision(
                                    reason="1/denom feeds bf16 softmax"):
                                nc.vector.reciprocal(recA[:, hh],
                                                     oA[HD:HD + 1, hh])
                                nc.vector.reciprocal(recB[:, hh],
                                                     oB[HD:HD + 1, hh])
                            # both broadcasts into one 1-bank ps2 tile
                            # (row-halves via output base partition) so the
                            # psb pool's y accumulators are not pinned
                            nc.tensor.matmul(rpAB[0:HD, hh], ones1[:],
                                             recA[:, hh],
                                             start=True, stop=True)
                            nc.tensor.matmul(rpAB[HD:2 * HD, hh], ones1[:],
                                             recB[:, hh],
                                             start=True, stop=True)
                            nc.vector.tensor_copy(rpSB[:, hh], rpAB[:, hh])
                        # normalization itself is chunked per t-tile and
                        # interleaved with the final output projection,
                        # multiplying straight out of the o psum tiles
                        _tailctx.update(oA=oA, oB=oB, rpSB=rpSB)
                        return
                    oU = tiny.tile([128, 512], F32, tag="oU",
                                   name=f"oU_{_it}_{p}_{G}")
                    if True:
                        # DRAM-bounce broadcast (latency hidden by the
                        # interleaved work); normalize on the idle Pool
                        # engine.  Order: finish all reads of oA before
                        # touching oB so each psum tile frees earlier.
                        recA = tiny.tile([1, 512], F32, tag="recAb",
                                         name=f"recAb_{_it}_{p}_{G}")
                        recB = tiny.tile([1, 512], F32, tag="recBb",
                                         name=f"recBb_{_it}_{p}_{G}")
                        nc.vector.tensor_copy(oU[0:HD, :], oA[0:HD, :])
                        nc.vector.reciprocal(recA[:], oA[HD:HD + 1, :])
                        nc.vector.tensor_copy(oU[HD:2 * HD, :], oB[0:HD, :])
                        nc.vector.reciprocal(recB[:], oB[HD:HD + 1, :])
                        rdr = dramp.tile([2, 512], F32)
                        nc.sync.dma_start(rdr[0:1, :], recA[:])
                        nc.sync.dma_start(rdr[1:2, :], recB[:])
                        Rsb = tiny.tile([128, 512], F32, tag="Rsb",
                                        name=f"Rsb_{_it}_{p}_{G}")
                        rdrap = rdr[:]
                        ap = list(rdrap.ap)
                        bcast = bass.AP(tensor=rdrap.tensor,
                                        offset=rdrap.offset,
                                        ap=[ap[0], [0, HD]] + ap[1:])
                        nc.sync.dma_start(Rsb[:], bcast)
                        nc.gpsimd.tensor_mul(oT[:, p, qsl], oU[:], Rsb[:])

                def attn_quarter(G, fillers):
                    """Attention for q-group G (both pairs) with the filler
                    units spread between attention tiles, weighted by each
                    tile's ACT-vs-PE deficit (diagonal tiles have tiny
                    matmuls but near-full-width exps)."""
                    njt = 4 * G + 4
                    n = 2 * njt
                    deficit = []
                    for p in range(NPAIR):
                        for j in range(njt):
                            off = max(0, j - 4 * G) * 128
                            pe = 4 * (512 - off) * 0.417
                            act = (1024 - off) * 0.833 + 185
                            d = max(60.0, act - pe)
                            if j == 0:
                                d += 400.0  # pair-start pipeline fill
                            deficit.append(d)
                    tot = sum(deficit)
                    cum, c = [], 0.0
                    for d in deficit:
                        c += d
                        cum.append(c)
                    fill_after = {}
                    m = max(1, len(fillers))
                    ti = 0
                    for i, fu in enumerate(fillers):
                        target = (i + 1) * tot / m
                        while ti < n - 1 and cum[ti] < target:
                            ti += 1
                        fill_after.setdefault(ti, []).append(fu)
                    t = 0
                    pending_av = None  # 1-tile software pipeline: emit tile
                    # j's AV after tile j+1's scores so the AV's exp
                    # dependency is already satisfied when it reaches the
                    # head of the PE queue.
                    for p in range(NPAIR):
                        hA, hB = 2 * p, 2 * p + 1
                        oA = psa.tile([HD + 1, 512], F32, tag="small",
                                      name=f"oA_{_it}_{p}_{G}")
                        oB = psa.tile([HD + 1, 512], F32, tag="small",
                                      name=f"oB_{_it}_{p}_{G}")
                        for j in range(njt):
                            dlt = j - 4 * G
                            off = max(0, dlt) * 128
                            qs = slice(512 * G + off, 512 * (G + 1))
                            ks = slice(128 * j, 128 * (j + 1))
                            # scores for both heads, one 2-bank psum tile
                            sAB = ps2.tile([128, 1024], F32, tag="big")
                            nc.tensor.matmul(sAB[:, off:512],
                                             kT[0:64, p, ks],
                                             qT[0:64, p, qs],
                                             start=True, stop=True)
                            nc.tensor.matmul(sAB[:, 512 + off:1024],
                                             kT[64:128, p, ks],
                                             qT[64:128, p, qs],
                                             start=True, stop=True)
                            pAB = workp.tile([128, 1024], BF, tag="pT")
                            if off > 0:
                                # diagonal: one strided exp over both heads'
                                # valid column blocks, skipping the dead span
                                # in between
                                s3 = sAB[:].rearrange(
                                    "p (two q) -> p two q", two=2)[:, :, off:]
                                p3 = pAB[:].rearrange(
                                    "p (two q) -> p two q", two=2)[:, :, off:]
                                nc.scalar.activation(p3, s3, AF.Exp)
                            else:
                                nc.scalar.activation(pAB[:, off:],
                                                     sAB[:, off:], AF.Exp)
                            if dlt >= 0:  # diagonal: multiplicative mask
                                dst = pAB[:].rearrange(
                                    "p (two q) -> p two q",
                                    two=2)[:, :, off:off + 128]
                                nc.vector.tensor_mul(
                                    dst, dst,
                                    mask01[:, None, :].to_broadcast(
                                        (128, 2, 128)))
                            # one filler sits between this tile's scores
                            # and the PREVIOUS tile's AV so the exp->mask
                            # chain latency hides behind it; extra fillers
                            # follow the AV
                            flist = fill_after.get(t, [])
                            for fu in flist[:1]:
                                fu()
                            if pending_av is not None:
                                pending_av()
                            for fu in flist[1:]:
                                fu()

                            def av(oA=oA, oB=oB, hA=hA, hB=hB, j=j, off=off,
                                   pAB=pAB, first=(j == 0),
                                   last=(j == njt - 1)):
                                nc.tensor.matmul(oA[:, off:],
                                                 v[:, j, hA, :],
                                                 pAB[:, off:512],
                                                 start=first, stop=last)
                                nc.tensor.matmul(oB[:, off:],
                                                 v[:, j, hB, :],
                                                 pAB[:, 512 + off:1024],
                                                 start=first, stop=last)
                            pending_av = av
                            t += 1
                        # drain the pipeline at the pair boundary (the norm
                        # needs the pair's last AV)
                        pending_av()
                        pending_av = None
                        attn_norm(p, G, oA, oB)

                # ---- the pipeline: quarter 0's projections up front, then
                # attention group G interleaved with quarter G+1's
                # projections and group G-1's output projection ----
                for h in (0, 1):
                    for fu in proj_units(h):
                        fu()
                # y-unit fillers go to the last, attention-heavy quarter
                # where the PE otherwise idles on the ACT's exp deficit
                y_fillers = {
                    2: [make_y_unit(tt, jh)
                        for tt in range(0, 4) for jh in range(2)],
                    3: [make_y_unit(tt, jh)
                        for tt in range(4, 12) for jh in range(2)],
                }
                for quar in range(4):
                    fillers = []
                    if quar < 3:
                        fillers += proj_units(2 * quar + 2)
                        fillers += proj_units(2 * quar + 3)
                    fillers += y_fillers.get(quar, [])
                    attn_quarter(quar, fillers)
                # final group: normalize oT one t-tile at a time, each chunk
                # immediately feeding its output-projection tile so the tail
                # pipelines instead of serializing norm -> all matmuls -> DMA
                oA3, oB3, rpSB = (_tailctx[k] for k in ("oA", "oB", "rpSB"))
                for tt in (15, 12, 13, 14):
                    ch = slice(128 * (tt - 12), 128 * (tt - 11))
                    ts = slice(128 * tt, 128 * (tt + 1))
                    nc.vector.tensor_mul(oT[0:HD, 1, ts], oA3[0:HD, ch],
                                         rpSB[0:HD, ch])
                    nc.vector.tensor_mul(oT[HD:2 * HD, 1, ts],
                                         oB3[0:HD, ch], rpSB[HD:2 * HD, ch])
                    ysb = worky.tile([128, 1024], BF, tag="ysbT",
                                     name=f"ysbT_{tt}_{_it}")
                    for jh in range(2):
                        acc = psb.tile([128, 512], F32, tag="pacc",
                                       name=f"yacc_{tt}_{jh}_{_it}")
                        for p in range(NPAIR):
                            nc.tensor.matmul(
                                acc[:],
                                oT[:, p, ts],
                                wot_sb[:, p, 512 * jh:512 * (jh + 1)],
                                start=(p == 0), stop=(p == NPAIR - 1),
                            )
                        # drain halves on DVE and ACT (ACT is idle by now)
                        if jh == 0:
                            nc.vector.tensor_copy(
                                ysb[:, 0:512], acc[:])
                        else:
                            nc.scalar.copy(ysb[:, 512:1024], acc[:])
                    if tt == 14:
                        nc.sync.dma_start(y_d[ts, 0:512], ysb[:, 0:512])
                        nc.sync.dma_start(y_d[ts, 512:1024], ysb[:, 512:1024])
                    else:
                        nc.sync.dma_start(y_d[ts, :], ysb[:])

    nc.compile()
    return nc


_RUNNER = None


def _make_pjrt_runner(nc):
    """Wrap a compiled Bass program as an 8-core PJRT callable."""
    import jax
    import numpy as _np
    from jax.sharding import Mesh, PartitionSpec
    from jax.experimental.shard_map import shard_map
    from concourse import bass2jax, mybir
    from concourse.bass2jax import (_bass_exec_p, install_neuronx_cc_hook,
                                    partition_id_tensor)

    install_neuronx_cc_hook()

    partition_name = (nc.partition_id_tensor.name
                      if nc.partition_id_tensor else None)
    in_names, out_names, out_avals = [], [], []
    for alloc in nc.m.functions[0].allocations:
        if not isinstance(alloc, mybir.MemoryLocationSet):
            continue
        if not alloc.memorylocations:
            continue
        name = alloc.memorylocations[0].name
        if alloc.kind == "ExternalInput":
            if name != partition_name:
                in_names.append(name)
        elif alloc.kind == "ExternalOutput":
            out_names.append(name)
            out_avals.append(jax.core.ShapedArray(
                tuple(alloc.tensor_shape), mybir.dt.np(alloc.dtype)))
    n_params = len(in_names)
    n_outs = len(out_names)
    zero_shapes = [(a.shape, a.dtype) for a in out_avals]
    all_in_names = in_names + out_names
    if partition_name is not None:
        all_in_names = all_in_names + [partition_name]

    def _body(*args):
        operands = list(args)
        if partition_name is not None:
            operands.append(partition_id_tensor())
        outs = _bass_exec_p.bind(
            *operands,
            out_avals=tuple(out_avals),
            in_names=tuple(all_in_names),
            out_names=tuple(out_names),
            lowering_input_output_aliases=(),
            sim_require_finite=True,
            sim_require_nnan=True,
            nc=nc,
        )
        return tuple(outs)

    devices = jax.devices()[:NCORES]
    mesh = Mesh(np.asarray(devices), ("core",))
    sharded = jax.jit(
        shard_map(_body, mesh=mesh,
                  in_specs=(PartitionSpec("core"),) * (n_params + n_outs),
                  out_specs=(PartitionSpec("core"),) * n_outs,
                  check_rep=False),
        keep_unused=True,
    )

    def run(in_maps):
        concat_in = [
            _np.concatenate([_np.asarray(in_maps[c][n]) for c in range(NCORES)],
                            axis=0)
            for n in in_names
        ]
        concat_zeros = [
            _np.zeros((NCORES * s[0], *s[1:]), d) for (s, d) in zero_shapes
        ]
        out_arrs = sharded(*concat_in, *concat_zeros)
        return [
            {
                n: _np.asarray(out_arrs[i]).reshape(NCORES, *out_avals[i].shape)[c]
                for i, n in enumerate(out_names)
            }
            for c in range(NCORES)
        ]

    internals = dict(nc=nc, body=_body, mesh=mesh, in_names=in_names,
                     out_names=out_names, zero_shapes=zero_shapes,
                     n_params=n_params)
    return run, in_names, internals


def _get_runner():
    """Build the Bass program once and return a cached 8-core PJRT callable."""
    global _RUNNER, _INTERNALS
    if _RUNNER is not None:
        return _RUNNER
    run, in_names, internals = _make_pjrt_runner(_build_program())
    _INTERNALS = internals
    _RUNNER = (run, in_names)
    return _RUNNER


def _make_in_maps(x, wq, wk, wv, wo):
    import ml_dtypes
    BF = ml_dtypes.bfloat16
    x = np.asarray(x, np.float32)
    wq_s = np.asarray(wq, np.float32) * (1.0 / np.sqrt(HD))  # fold score scale
    wk = np.asarray(wk, np.float32)
    wv = np.asarray(wv, np.float32)
    wo = np.asarray(wo, np.float32)

    xt_b = [np.ascontiguousarray(x[b].T).astype(BF) for b in range(B)]
    in_maps = []
    for c in range(NCORES):
        b, g = c // GROUPS, c % GROUPS
        sl = slice(DH * g, DH * (g + 1))
        in_maps.append({
            "xt": xt_b[b],
            "wqt": np.ascontiguousarray(wq_s[sl, :].T).astype(BF),
            "wkt": np.ascontiguousarray(wk[sl, :].T).astype(BF),
            "wvt": np.ascontiguousarray(wv[sl, :].T).astype(BF),
            "wot": np.ascontiguousarray(wo[:, sl].T).astype(BF),
        })
    return in_maps


def kernel(x, wq, wk, wv, wo):
    run, _ = _get_runner()
    results = run(_make_in_maps(x, wq, wk, wv, wo))
    y = np.zeros((B, T, DIM), np.float32)
    for c in range(NCORES):
        y[c // GROUPS] += results[c]["y"].astype(np.float32)
    return y


# revision 58
# speedup vs baseline: 1.0045x; 1.0045x over previous
"""Causal multi-head attention (B=2, T=2048, DIM=1024, H=16) on 8 TRN2 cores.

Sharding: core c handles batch b = c // 4 and head-group g = c % 4 (4 heads,
head-dim slice of 256).  Each core computes QKV projections for its heads,
causal attention, and a partial output projection y_partial = o_g @ wo[:, g].T
of shape (2048, 1024).  Host sums the 4 partials per batch (the tensor-parallel
all-reduce, done on host as the unshard step).

All matmuls run in bfloat16 (full PE rate at any tile size; fp32r pays a 4x
penalty below 256 moving columns, which hits the causal diagonal tiles).
Inputs are pre-rounded to bf16 on host; end-to-end error vs the fp32
reference is ~2e-3, well inside the 2e-2 gate.

The kernel is a software pipeline over the 4 token-quarters so the scalar
(ACT) engine's exp work -- the single largest non-PE cost, ~63us -- overlaps
the PE-only projection and output-projection phases instead of running after
them:

    [proj Q0 | attn G0 | proj Q1 | y G0 | attn G1 | proj Q2 | y G1 | ...]

Attention group G only needs k/v tiles 0..4G+3, i.e. quarters <= G, so each
group runs as soon as its quarter is projected.  Projections are emitted at
half-quarter (256-token) granularity so the first matmul starts ~3us after
launch instead of waiting for a full 2MB x-quarter DMA.

Device layout (T=2048 tokens of one batch, DH=256 head dims of one group):
  xsb  [128, 8, T]   x with the DIM contraction split into 8 chunks of 128
  qT/kT[128, 2, T]   per pair p of 2 heads; partitions = 2x64 head dims
  v    [128, 16, 4, 65]  [t-tile, k-in-tile, head, head-dim + ones column]
  scores sT[k, q] via matmul(lhsT=kT, rhs=qT); softmax without max-subtraction
  (scores ~N(0,1)); denominator accumulated by the ones column of v during
  attn@v; normalization applied via a DRAM-bounce broadcast of 1/denom
  (PE outer-product broadcast for the final, tail-critical pair).
"""

import sys

sys.path.insert(0, "/opt/trn_rl_repo")

import numpy as np

B, T, DIM, H = 2, 2048, 1024, 16
HD = DIM // H          # 64
NCORES = 8
GROUPS = 4             # head-groups (one per core pair-of-batches)
GH = H // GROUPS       # 4 heads per group
DH = GH * HD           # 256 head dims per group
NPAIR = 2              # pairs of heads per group (2 heads = 128 partitions)
TT = T // 128          # 16 t-tiles
TG = T // 512          # 4 q-groups of 512


def _build_program(loop=1):
    import concourse.bass as bass
    import concourse.tile as tile
    from concourse import bacc, mybir

    F32 = mybir.dt.float32
    BF = mybir.dt.bfloat16
    AF = mybir.ActivationFunctionType

    nc = bacc.Bacc("TRN2", target_bir_lowering=False, debug=False,
                   num_devices=NCORES)

    xt_d = nc.dram_tensor("xt", [DIM, T], BF, kind="ExternalInput")
    wqt_d = nc.dram_tensor("wqt", [DIM, DH], BF, kind="ExternalInput")
    wkt_d = nc.dram_tensor("wkt", [DIM, DH], BF, kind="ExternalInput")
    wvt_d = nc.dram_tensor("wvt", [DIM, DH], BF, kind="ExternalInput")
    wot_d = nc.dram_tensor("wot", [DH, DIM], BF, kind="ExternalInput")
    y_d = nc.dram_tensor("y", [T, DIM], BF, kind="ExternalOutput")

    KO = DIM // 128  # 8 contraction chunks

    with tile.TileContext(nc) as tc:
        with (
            tc.tile_pool(name="singles", bufs=1) as singles,
            tc.tile_pool(name="workp", bufs=6) as workp,
            tc.tile_pool(name="worky", bufs=6) as worky,
            tc.tile_pool(name="tiny", bufs=4) as tiny,
            tc.tile_pool(name="ps2", bufs=2, space="PSUM") as ps2,
            tc.tile_pool(name="psa", bufs=2, space="PSUM") as psa,
            tc.tile_pool(name="psb", bufs=2, space="PSUM") as psb,
            tc.tile_pool(name="dramp", bufs=2, space="DRAM") as dramp,
        ):
            # ---- persistent SBUF tensors ----
            qT = singles.tile([128, NPAIR, T], BF)
            kT = singles.tile([128, NPAIR, T], BF)
            v = singles.tile([128, TT, GH, HD + 1], BF)
            oT = singles.tile([128, NPAIR, T], BF)

            mask01 = singles.tile([128, 128], BF)
            nc.gpsimd.memset(mask01[:], 1.0)
            # keep 1 where q - k >= 0 (k on partitions, q on free), else 0
            nc.gpsimd.affine_select(
                out=mask01[:], in_=mask01[:],
                compare_op=mybir.AluOpType.is_ge, fill=0.0,
                base=0, pattern=[[1, 128]], channel_multiplier=-1,
            )
            ones1 = singles.tile([1, HD], BF)
            nc.vector.memset(ones1[:], 1.0)
            # ones column of v (denominator accumulator)
            for hh in range(GH):
                nc.vector.memset(v[:, :, hh, HD:HD + 1], 1.0)
            # warm the ACT exp table during the initial DMA
            dummyf = singles.tile([128, 1], F32)
            nc.vector.memset(dummyf[:], 1.0)
            dummy = singles.tile([128, 1], F32)
            nc.scalar.activation(dummy[:], dummyf[:], AF.Exp)
            # warm the PE p-state during the initial DMA wait: the tensor
            # engine clock ramps with sustained use (full speed after ~3us),
            # so burn the dead time before the first weights arrive on
            # throwaway matmuls
            wrow = singles.tile([1, 512], BF)
            nc.vector.memset(wrow[:], 0.0)
            wps = psb.tile([HD, 512], F32, tag="pacc", name="pewarm")
            for _w in range(6):
                nc.tensor.matmul(wps[:], ones1[:], wrow[:],
                                 start=True, stop=True)

            # ---- device-side repetition for timing (loop > 1) ----
            for _it in range(loop):
              with (tc.tile_pool(name=f"wpool{_it}", bufs=1) as wpool,):
                xt_r = xt_d.rearrange("(ko p) t -> p ko t", p=128)
                wqt_sb = wpool.tile([128, KO, DH], BF)
                wkt_sb = wpool.tile([128, KO, DH], BF)
                wvt_sb = wpool.tile([128, KO, DH], BF)
                wot_sb = wpool.tile([128, DH // 128, DIM], BF)
                xsb = wpool.tile([128, KO, T], BF)
                wqt_r = wqt_d.rearrange("(ko p) d -> p ko d", p=128)

                # DMA order = need order: wqt + first x half gate the first
                # matmul; later halves stream in behind the compute.
                nc.sync.dma_start(wqt_sb[:, 0:4, :], wqt_r[:, 0:4, :])
                nc.sync.dma_start(xsb[:, 0:2, 0:256], xt_r[:, 0:2, 0:256])
                nc.sync.dma_start(xsb[:, 2:4, 0:256], xt_r[:, 2:4, 0:256])
                nc.sync.dma_start(wqt_sb[:, 4:8, :], wqt_r[:, 4:8, :])
                nc.sync.dma_start(xsb[:, 4:6, 0:256], xt_r[:, 4:6, 0:256])
                nc.sync.dma_start(xsb[:, 6:8, 0:256], xt_r[:, 6:8, 0:256])
                nc.sync.dma_start(wkt_sb,
                                  wkt_d.rearrange("(ko p) d -> p ko d", p=128))
                nc.sync.dma_start(wvt_sb,
                                  wvt_d.rearrange("(ko p) d -> p ko d", p=128))
                for h in range(1, 8):
                    hs = slice(256 * h, 256 * (h + 1))
                    nc.sync.dma_start(xsb[:, :, hs], xt_r[:, :, hs])
                nc.sync.dma_start(wot_sb,
                                  wot_d.rearrange("(ko p) j -> p ko j", p=128))

                _tailctx = {}

                # ---- filler work units (one unit ~ 850ns of PE time) ----
                # Emitted between attention tiles so the PE fills the per-tile
                # deficit vs the ACT engine's exp (which is ~20% slower per
                # attention tile than the PE's scores+AV matmuls).
                _qk_acc = {}

                def make_qk_unit(w_sb, dst, h, p, key):
                    hs = slice(256 * h, 256 * (h + 1))

                    def f():
                        if p == 0:
                            _qk_acc[key] = psb.tile(
                                [128, 512], F32, tag="pacc",
                                name=f"acc_{key}_{_it}")
                        acc = _qk_acc[key]
                        for ko in range(KO):
                            nc.tensor.matmul(
                                acc[:, 256 * p:256 * (p + 1)],
                                w_sb[:, ko, 128 * p:128 * (p + 1)],
                                xsb[:, ko, hs],
                                start=(ko == 0), stop=(ko == KO - 1),
                            )
                        if p == NPAIR - 1:
                            nc.vector.tensor_copy(
                                dst[:, :, hs],
                                acc[:].rearrange("par (p t) -> par p t",
                                                 p=NPAIR))
                    return f

                def make_v_unit(tt):
                    def f():
                        acc = psb.tile([128, DH], F32, tag="pacc",
                                       name=f"vacc_{tt}_{_it}")
                        for ko in range(KO):
                            nc.tensor.matmul(
                                acc[:],
                                xsb[:, ko, 128 * tt:128 * (tt + 1)],
                                wvt_sb[:, ko, :],
                                start=(ko == 0), stop=(ko == KO - 1),
                            )
                        nc.vector.tensor_copy(
                            v[:, tt, :, 0:HD],
                            acc[:].rearrange("p (h d) -> p h d", h=GH))
                    return f

                def proj_units(h):
                    """Six ~850ns units projecting tokens [256h, 256h+256)."""
                    return [
                        make_qk_unit(wqt_sb, qT, h, 0, f"q{h}"),
                        make_qk_unit(wqt_sb, qT, h, 1, f"q{h}"),
                        make_qk_unit(wkt_sb, kT, h, 0, f"k{h}"),
                        make_qk_unit(wkt_sb, kT, h, 1, f"k{h}"),
                        make_v_unit(2 * h),
                        make_v_unit(2 * h + 1),
                    ]

                def make_y_unit(tt, jh, drain_eng=None):
                    """Half an output-projection tile: 1-bank psum, drained
                    on DVE so the y work never queues behind pending exps on
                    ACT (which would stall the shared psum rotation)."""
                    def f():
                        acc = psb.tile([128, 512], F32, tag="pacc",
                                       name=f"yacc_{tt}_{jh}_{_it}")
                        for p in range(NPAIR):
                            nc.tensor.matmul(
                                acc[:],
                                oT[:, p, 128 * tt:128 * (tt + 1)],
                                wot_sb[:, p, 512 * jh:512 * (jh + 1)],
                                start=(p == 0), stop=(p == NPAIR - 1),
                            )
                        ysb = worky.tile([128, 512], BF, tag="ysb")
                        if drain_eng is None:
                            nc.vector.tensor_copy(ysb[:], acc[:])
                        else:
                            drain_eng.copy(ysb[:], acc[:])
                        nc.sync.dma_start(
                            y_d[128 * tt:128 * (tt + 1),
                                512 * jh:512 * (jh + 1)], ysb[:])
                    return f

                def attn_norm(p, G, oA, oB):
                    """Softmax denominator normalization for pair p of group
                    G: o /= denom with 1/denom broadcast across partitions."""
                    qsl = slice(512 * G, 512 * (G + 1))
                    last = (p == NPAIR - 1 and G == TG - 1)
                    if last:
                        # reciprocals first: they gate the PE broadcast.
                        # Everything is halved so the first tail mul starts
                        # one DVE-op-latency earlier.
                        recA = tiny.tile([1, 512], BF, tag="recA")
                        recB = tiny.tile([1, 512], BF, tag="recB")
                        rpAB = ps2.tile([128, 512], F32, tag="big",
                                        name=f"rpAB_{_it}")
                        rpSB = tiny.tile([128, 512], F32, tag="rpSB")
                        for hh in (slice(256, 512), slice(0, 256)):
                            with nc.allow_low_precision(
                                    reason="1/denom feeds bf16 softmax"):
                                nc.vector.reciprocal(recA[:, hh],
                                                     oA[HD:HD + 1, hh])
                                nc.vector.reciprocal(recB[:, hh],
                                                     oB[HD:HD + 1, hh])
                            # both broadcasts into one 1-bank ps2 tile
                            # (row-halves via output base partition) so the
                            # psb pool's y accumulators are not pinned
                            nc.tensor.matmul(rpAB[0:HD, hh], ones1[:],
                                             recA[:, hh],
                                             start=True, stop=True)
                            nc.tensor.matmul(rpAB[HD:2 * HD, hh], ones1[:],
                                             recB[:, hh],
                                             start=True, stop=True)
                            nc.vector.tensor_copy(rpSB[:, hh], rpAB[:, hh])
                        # normalization itself is chunked per t-tile and
                        # interleaved with the final output projection,
                        # multiplying straight out of the o psum tiles
                        _tailctx.update(oA=oA, oB=oB, rpSB=rpSB)
                        return
                    oU = tiny.tile([128, 512], F32, tag="oU",
                                   name=f"oU_{_it}_{p}_{G}")
                    if True:
                        # DRAM-bounce broadcast (latency hidden by the
                        # interleaved work); normalize on the idle Pool
                        # engine.  Order: finish all reads of oA before
                        # touching oB so each psum tile frees earlier.
                        recA = tiny.tile([1, 512], F32, tag="recAb",
                                         name=f"recAb_{_it}_{p}_{G}")
                        recB = tiny.tile([1, 512], F32, tag="recBb",
                                         name=f"recBb_{_it}_{p}_{G}")
                        nc.vector.tensor_copy(oU[0:HD, :], oA[0:HD, :])
                        nc.vector.reciprocal(recA[:], oA[HD:HD + 1, :])
                        nc.vector.tensor_copy(oU[HD:2 * HD, :], oB[0:HD, :])
                        nc.vector.reciprocal(recB[:], oB[HD:HD + 1, :])
                        rdr = dramp.tile([2, 512], F32)
                        nc.sync.dma_start(rdr[0:1, :], recA[:])
                        nc.sync.dma_start(rdr[1:2, :], recB[:])
                        Rsb = tiny.tile([128, 512], F32, tag="Rsb",
                                        name=f"Rsb_{_it}_{p}_{G}")
                        rdrap = rdr[:]
                        ap = list(rdrap.ap)
                        bcast = bass.AP(tensor=rdrap.tensor,
                                        offset=rdrap.offset,
                                        ap=[ap[0], [0, HD]] + ap[1:])
                        nc.sync.dma_start(Rsb[:], bcast)
                        nc.gpsimd.tensor_mul(oT[:, p, qsl], oU[:], Rsb[:])

                def attn_quarter(G, fillers):
                    """Attention for q-group G (both pairs) with the filler
                    units spread between attention tiles, weighted by each
                    tile's ACT-vs-PE deficit (diagonal tiles have tiny
                    matmuls but near-full-width exps)."""
                    njt = 4 * G + 4
                    n = 2 * njt
                    deficit = []
                    for p in range(NPAIR):
                        for j in range(njt):
                            off = max(0, j - 4 * G) * 128
                            pe = 4 * (512 - off) * 0.417
                            act = (1024 - off) * 0.833 + 185
                            d = max(60.0, act - pe)
                            if j == 0:
                                d += 400.0  # pair-start pipeline fill
                            deficit.append(d)
                    tot = sum(deficit)
                    cum, c = [], 0.0
                    for d in deficit:
                        c += d
                        cum.append(c)
                    fill_after = {}
                    m = max(1, len(fillers))
                    ti = 0
                    for i, fu in enumerate(fillers):
                        target = (i + 1) * tot / m
                        while ti < n - 1 and cum[ti] < target:
                            ti += 1
                        fill_after.setdefault(ti, []).append(fu)
                    t = 0
                    pending_av = None  # 1-tile software pipeline: emit tile
                    # j's AV after tile j+1's scores so the AV's exp
                    # dependency is already satisfied when it reaches the
                    # head of the PE queue.
                    for p in range(NPAIR):
                        hA, hB = 2 * p, 2 * p + 1
                        oA = psa.tile([HD + 1, 512], F32, tag="small",
                                      name=f"oA_{_it}_{p}_{G}")
                        oB = psa.tile([HD + 1, 512], F32, tag="small",
                                      name=f"oB_{_it}_{p}_{G}")
                        for j in range(njt):
                            dlt = j - 4 * G
                            off = max(0, dlt) * 128
                            qs = slice(512 * G + off, 512 * (G + 1))
                            ks = slice(128 * j, 128 * (j + 1))
                            # scores for both heads, one 2-bank psum tile
                            sAB = ps2.tile([128, 1024], F32, tag="big")
                            nc.tensor.matmul(sAB[:, off:512],
                                             kT[0:64, p, ks],
                                             qT[0:64, p, qs],
                                             start=True, stop=True)
                            nc.tensor.matmul(sAB[:, 512 + off:1024],
                                             kT[64:128, p, ks],
                                             qT[64:128, p, qs],
                                             start=True, stop=True)
                            pAB = workp.tile([128, 1024], BF, tag="pT")
                            if off > 0:
                                # diagonal: one strided exp over both heads'
                                # valid column blocks, skipping the dead span
                                # in between
                                s3 = sAB[:].rearrange(
                                    "p (two q) -> p two q", two=2)[:, :, off:]
                                p3 = pAB[:].rearrange(
                                    "p (two q) -> p two q", two=2)[:, :, off:]
                                nc.scalar.activation(p3, s3, AF.Exp)
                            else:
                                nc.scalar.activation(pAB[:, off:],
                                                     sAB[:, off:], AF.Exp)
                            if dlt >= 0:  # diagonal: multiplicative mask
                                dst = pAB[:].rearrange(
                                    "p (two q) -> p two q",
                                    two=2)[:, :, off:off + 128]
                                nc.vector.tensor_mul(
                                    dst, dst,
                                    mask01[:, None, :].to_broadcast(
                                        (128, 2, 128)))
                            # one filler sits between this tile's scores
                            # and the PREVIOUS tile's AV so the exp->mask
                            # chain latency hides behind it; extra fillers
                            # follow the AV
                            flist = fill_after.get(t, [])
                            for fu in flist[:1]:
                                fu()
                            if pending_av is not None:
                                pending_av()
                            for fu in flist[1:]:
                                fu()

                            def av(oA=oA, oB=oB, hA=hA, hB=hB, j=j, off=off,
                                   pAB=pAB, first=(j == 0),
                                   last=(j == njt - 1)):
                                nc.tensor.matmul(oA[:, off:],
                                                 v[:, j, hA, :],
                                                 pAB[:, off:512],
                                                 start=first, stop=last)
                                nc.tensor.matmul(oB[:, off:],
                                                 v[:, j, hB, :],
                                                 pAB[:, 512 + off:1024],
                                                 start=first, stop=last)
                            pending_av = av
                            t += 1
                        # drain the pipeline at the pair boundary (the norm
                        # needs the pair's last AV)
                        pending_av()
                        pending_av = None
                        attn_norm(p, G, oA, oB)

                # ---- the pipeline: quarter 0's projections up front, then
                # attention group G interleaved with quarter G+1's
                # projections and group G-1's output projection ----
                for h in (0, 1):
                    for fu in proj_units(h):
                        fu()
                # y-unit fillers go to the last, attention-heavy quarter
                # where the PE otherwise idles on the ACT's exp deficit
                y_fillers = {
                    2: [make_y_unit(tt, jh)
                        for tt in range(0, 4) for jh in range(2)],
                    3: [make_y_unit(tt, jh)
                        for tt in range(4, 12) for jh in range(2)],
                }
                held = []
                for quar in range(4):
                    fillers = []
                    if quar < 3:
                        fillers += proj_units(2 * quar + 2)
                        fillers += proj_units(2 * quar + 3)
                    fillers += y_fillers.get(quar, [])
                    if quar == 3:
                        held = fillers[-2:]
                        fillers = fillers[:-2]
                    attn_quarter(quar, fillers)
                # two held-back y units fill the PE while the final pair's
                # normalization chain runs on DVE
                for fu in held:
                    fu()
                # final group: normalize oT one t-tile at a time, each chunk
                # immediately feeding its output-projection tile so the tail
                # pipelines instead of serializing norm -> all matmuls -> DMA
                oA3, oB3, rpSB = (_tailctx[k] for k in ("oA", "oB", "rpSB"))
                for tt in (15, 12, 13, 14):
                    ch = slice(128 * (tt - 12), 128 * (tt - 11))
                    ts = slice(128 * tt, 128 * (tt + 1))
                    nc.vector.tensor_mul(oT[0:HD, 1, ts], oA3[0:HD, ch],
                                         rpSB[0:HD, ch])
                    nc.vector.tensor_mul(oT[HD:2 * HD, 1, ts],
                                         oB3[0:HD, ch], rpSB[HD:2 * HD, ch])
                    ysb = worky.tile([128, 1024], BF, tag="ysbT",
                                     name=f"ysbT_{tt}_{_it}")
                    for jh in range(2):
                        acc = psb.tile([128, 512], F32, tag="pacc",
                                       name=f"yacc_{tt}_{jh}_{_it}")
                        for p in range(NPAIR):
                            nc.tensor.matmul(
                                acc[:],
                                oT[:, p, ts],
                                wot_sb[:, p, 512 * jh:512 * (jh + 1)],
                                start=(p == 0), stop=(p == NPAIR - 1),
                            )
                        # drain halves on DVE and ACT (ACT is idle by now)
                        if jh == 0:
                            nc.vector.tensor_copy(
                                ysb[:, 0:512], acc[:])
                        else:
                            nc.scalar.copy(ysb[:, 512:1024], acc[:])
                    if tt == 14:
                        nc.sync.dma_start(y_d[ts, 0:512], ysb[:, 0:512])
                        nc.sync.dma_start(y_d[ts, 512:1024], ysb[:, 512:1024])
                    else:
                        nc.sync.dma_start(y_d[ts, :], ysb[:])

    nc.compile()
    return nc


_RUNNER = None


def _make_pjrt_runner(nc):
    """Wrap a compiled Bass program as an 8-core PJRT callable."""
    import jax
    import numpy as _np
    from jax.sharding import Mesh, PartitionSpec
    from jax.experimental.shard_map import shard_map
    from concourse import bass2jax, mybir
    from concourse.bass2jax import (_bass_exec_p, install_neuronx_cc_hook,
                                    partition_id_tensor)

    install_neuronx_cc_hook()

    partition_name = (nc.partition_id_tensor.name
                      if nc.partition_id_tensor else None)
    in_names, out_names, out_avals = [], [], []
    for alloc in nc.m.functions[0].allocations:
        if not isinstance(alloc, mybir.MemoryLocationSet):
            continue
        if not alloc.memorylocations:
            continue
        name = alloc.memorylocations[0].name
        if alloc.kind == "ExternalInput":
            if name != partition_name:
                in_names.append(name)
        elif alloc.kind == "ExternalOutput":
            out_names.append(name)
            out_avals.append(jax.core.ShapedArray(
                tuple(alloc.tensor_shape), mybir.dt.np(alloc.dtype)))
    n_params = len(in_names)
    n_outs = len(out_names)
    zero_shapes = [(a.shape, a.dtype) for a in out_avals]
    all_in_names = in_names + out_names
    if partition_name is not None:
        all_in_names = all_in_names + [partition_name]

    def _body(*args):
        operands = list(args)
        if partition_name is not None:
            operands.append(partition_id_tensor())
        outs = _bass_exec_p.bind(
            *operands,
            out_avals=tuple(out_avals),
            in_names=tuple(all_in_names),
            out_names=tuple(out_names),
            lowering_input_output_aliases=(),
            sim_require_finite=True,
            sim_require_nnan=True,
            nc=nc,
        )
        return tuple(outs)

    devices = jax.devices()[:NCORES]
    mesh = Mesh(np.asarray(devices), ("core",))
    sharded = jax.jit(
        shard_map(_body, mesh=mesh,
                  in_specs=(PartitionSpec("core"),) * (n_params + n_outs),
                  out_specs=(PartitionSpec("core"),) * n_outs,
                  check_rep=False),
        keep_unused=True,
    )

    def run(in_maps):
        concat_in = [
            _np.concatenate([_np.asarray(in_maps[c][n]) for c in range(NCORES)],
                            axis=0)
            for n in in_names
        ]
        concat_zeros = [
            _np.zeros((NCORES * s[0], *s[1:]), d) for (s, d) in zero_shapes
        ]
        out_arrs = sharded(*concat_in, *concat_zeros)
        return [
            {
                n: _np.asarray(out_arrs[i]).reshape(NCORES, *out_avals[i].shape)[c]
                for i, n in enumerate(out_names)
            }
            for c in range(NCORES)
        ]

    internals = dict(nc=nc, body=_body, mesh=mesh, in_names=in_names,
                     out_names=out_names, zero_shapes=zero_shapes,
                     n_params=n_params)
    return run, in_names, internals


def _get_runner():
    """Build the Bass program once and return a cached 8-core PJRT callable."""
    global _RUNNER, _INTERNALS
    if _RUNNER is not None:
        return _RUNNER
    run, in_names, internals = _make_pjrt_runner(_build_program())
    _INTERNALS = internals
    _RUNNER = (run, in_names)
    return _RUNNER


def _make_in_maps(x, wq, wk, wv, wo):
    import ml_dtypes
    BF = ml_dtypes.bfloat16
    x = np.asarray(x, np.float32)
    wq_s = np.asarray(wq, np.float32) * (1.0 / np.sqrt(HD))  # fold score scale
    wk = np.asarray(wk, np.float32)
    wv = np.asarray(wv, np.float32)
    wo = np.asarray(wo, np.float32)

    xt_b = [np.ascontiguousarray(x[b].T).astype(BF) for b in range(B)]
    in_maps = []
    for c in range(NCORES):
        b, g = c // GROUPS, c % GROUPS
        sl = slice(DH * g, DH * (g + 1))
        in_maps.append({
            "xt": xt_b[b],
            "wqt": np.ascontiguousarray(wq_s[sl, :].T).astype(BF),
            "wkt": np.ascontiguousarray(wk[sl, :].T).astype(BF),
            "wvt": np.ascontiguousarray(wv[sl, :].T).astype(BF),
            "wot": np.ascontiguousarray(wo[:, sl].T).astype(BF),
        })
    return in_maps


def kernel(x, wq, wk, wv, wo):
    run, _ = _get_runner()
    results = run(_make_in_maps(x, wq, wk, wv, wo))
    y = np.zeros((B, T, DIM), np.float32)
    for c in range(NCORES):
        y[c // GROUPS] += results[c]["y"].astype(np.float32)
    return y
